# revision 3
# baseline (speedup 1.0000x reference)
"""Trainium2 kernel for the point-transformer backbone (nn_Backbone_71064528879926).

Strategy:
- Farthest-point sampling (the serial 1360-iteration argmax recurrence, which
  dominates and which XLA handles terribly) runs as a hand-written Bass/Tile
  kernel on a NeuronCore: all 4 point clouds' FPS run in one instruction
  stream (batch b on partition block 32b..32b+31), with the per-iteration
  argmax + winner-coordinate extraction done via an is_equal-mask multiply
  into an interleaved {index,-x,-y} payload and 32x32 block-transpose
  reductions (~13 DVE/ACT instructions per iteration, no gathers in the
  loop). Distance updates replicate the jax reference bit-for-bit (verified:
  ACT Square is exact fp32), so the sampled index sequence matches exactly.
- The dense remainder (KNN + vector attention + transition-down convs) runs
  on the same NeuronCores via XLA (jax jit), batch-parallel.

The full inputs are taken, outputs returned at full shape; float32 in/out.
"""

import numpy as np
import jax
import jax.numpy as jnp
from jax import lax

import concourse.bass as bass
import concourse.mybir as mybir
import concourse.tile as tile
from concourse import bass2jax

F32 = mybir.dt.float32
I32 = mybir.dt.int32
ALU = mybir.AluOpType
ACTF = mybir.ActivationFunctionType
AX = mybir.AxisListType

B, N0, FEAT_DIM = 4, 4096, 8
NBLOCKS, K, D_MODEL = 4, 16, 256


# ---------------------------------------------------------------------------
# walrus workaround: this container's walrus rejects >1 sync wait per
# instruction; move excess waits onto inserted single-wait NoOps.
# ---------------------------------------------------------------------------
_wfx_cnt = [0]


def _fix_sync_waits(nc, max_waits=1):
    for f in nc.m.functions:
        for bb in f.blocks:
            insts = bb.instructions
            if not any(
                ins.sync_info and ins.sync_info.on_wait
                and len(ins.sync_info.on_wait) > max_waits
                for ins in insts
            ):
                continue
            out = []
            for ins in insts:
                si = ins.sync_info
                if si and si.on_wait and len(si.on_wait) > max_waits:
                    waits = list(si.on_wait)
                    for w in waits[:-max_waits]:
                        nop = mybir.InstNoOp(
                            name=f"I-wfx-{_wfx_cnt[0]}", ins=[], outs=[])
                        _wfx_cnt[0] += 1
                        nop.engine = ins.engine
                        nop.sync_info = mybir.SyncInfo(on_wait=[w], on_update=[])
                        out.append(nop)
                    ins.sync_info = mybir.SyncInfo(
                        on_wait=waits[-max_waits:],
                        on_update=list(si.on_update or []))
                out.append(ins)
            bb.instructions = out


# ---------------------------------------------------------------------------
# jit-once PJRT runner for a bass module (single core)
# ---------------------------------------------------------------------------
def _make_runner(nc):
    bass2jax.install_neuronx_cc_hook()
    partition_name = (nc.partition_id_tensor.name
                      if nc.partition_id_tensor else None)
    in_names, out_names, out_avals, zero_outs = [], [], [], []
    for alloc in nc.m.functions[0].allocations:
        if not isinstance(alloc, mybir.MemoryLocationSet):
            continue
        name = alloc.memorylocations[0].name
        if alloc.kind == "ExternalInput":
            if name != partition_name:
                in_names.append(name)
        elif alloc.kind == "ExternalOutput":
            out_names.append(name)
            shape = tuple(alloc.tensor_shape)
            dtype = mybir.dt.np(alloc.dtype)
            out_avals.append(jax.core.ShapedArray(shape, dtype))
            zero_outs.append(np.zeros(shape, dtype))
    n_params = len(in_names)
    n_outs = len(out_avals)
    all_in = list(in_names) + list(out_names)
    if partition_name is not None:
        all_in.append(partition_name)

    def _body(*args):
        operands = list(args)
        if partition_name is not None:
            operands.append(bass2jax.partition_id_tensor())
        return tuple(bass2jax._bass_exec_p.bind(
            *operands, out_avals=tuple(out_avals), in_names=tuple(all_in),
            out_names=tuple(out_names), lowering_input_output_aliases=(),
            sim_require_finite=False, sim_require_nnan=False, nc=nc))

    donate = tuple(range(n_params, n_params + n_outs))
    fn = jax.jit(_body, donate_argnums=donate, keep_unused=True)

    def run(in_map):
        ins = [np.asarray(in_map[name]) for name in in_names]
        outs = fn(*ins, *[z.copy() for z in zero_outs])
        return dict(zip(out_names, [np.asarray(o) for o in outs]))

    return run


# ---------------------------------------------------------------------------
# FPS bass kernel: one level, all 4 batches at once
# ---------------------------------------------------------------------------
def _build_fps_runner(N):
    npoint = N // 4
    P, F = 128, N // 32
    nc = bass.Bass("TRN2", target_bir_lowering=False, debug=False,
                   num_devices=1)
    xy_in = nc.dram_tensor("xy", [B, N, 2], F32, kind="ExternalInput")
    fi_out = nc.dram_tensor("fi", [B, npoint], F32, kind="ExternalOutput")

    with tile.TileContext(nc) as tc:
        with tc.tile_pool(name="p", bufs=2) as pool, \
             tc.tile_pool(name="big", bufs=1) as big:
            xtbl = big.tile([P, F], F32, tag="xtbl")
            ytbl = big.tile([P, F], F32, tag="ytbl")
            xy3 = xy_in.ap().rearrange("b (pl f) two -> (b pl) f two", pl=32)
            nc.sync.dma_start(xtbl[:], xy3[:, :, 0:1].squeeze())
            nc.sync.dma_start(ytbl[:], xy3[:, :, 1:2].squeeze())

            iota_i = big.tile([P, F], I32, tag="iota_i")
            nc.gpsimd.iota(iota_i[:], pattern=[[1, F]], base=0,
                           channel_multiplier=F)
            iota_f = big.tile([P, F], F32, tag="iota_f")
            nc.vector.tensor_copy(iota_f[:], iota_i[:])
            bbase = big.tile([P, 1], F32, tag="bbase")
            for b in range(B):
                nc.vector.memset(bbase[32 * b:32 * (b + 1), :], float(b * N))
            nc.vector.tensor_tensor(iota_f[:], iota_f[:],
                                    bbase[:, 0:1].broadcast_to((P, F)),
                                    ALU.subtract)
            payload3 = big.tile([P, 3 * F], F32, tag="payload3")
            pv = payload3[:].rearrange("p (f k) -> p k f", k=3)
            nc.vector.tensor_copy(pv[:, 0, :], iota_f[:])
            nc.vector.tensor_scalar_mul(pv[:, 1, :], xtbl[:], -1.0)
            nc.vector.tensor_scalar_mul(pv[:, 2, :], ytbl[:], -1.0)
            out3pad = big.tile([P, 32], F32, tag="out3pad")
            nc.vector.memset(out3pad[:], 0.0)

            wbuf = big.tile([P, npoint], F32, tag="wbuf")
            nc.vector.memset(wbuf[:, 0:1], 0.0)

            # --- the FPS loop ---
            d = big.tile([P, F], F32, tag="fps_d")
            nc.vector.memset(d[:], 1e10)
            vals = pool.tile([P, 32], F32, tag="fps_vals")
            tr_nx = pool.tile([P, 32], F32, tag="fps_trnx")
            nc.vector.transpose(tr_nx[:],
                                payload3[:, 1:2].broadcast_to((P, 32)))
            tr_ny = pool.tile([P, 32], F32, tag="fps_trny")
            nc.vector.transpose(tr_ny[:],
                                payload3[:, 2:3].broadcast_to((P, 32)))
            nc.vector.tensor_copy(vals[:, 1:2], tr_nx[:, 0:1])
            nc.vector.tensor_copy(vals[:, 2:3], tr_ny[:, 0:1])

            for t in range(1, npoint):
                a = pool.tile([P, F], F32, tag="fps_a")
                bt = pool.tile([P, F], F32, tag="fps_b")
                nc.scalar.activation(a[:], xtbl[:], ACTF.Square,
                                     bias=vals[:, 1:2], scale=1.0)
                nc.scalar.activation(bt[:], ytbl[:], ACTF.Square,
                                     bias=vals[:, 2:3], scale=1.0)
                s = pool.tile([P, F], F32, tag="fps_s")
                nc.vector.tensor_tensor(s[:], a[:], bt[:], ALU.add)
                nc.vector.tensor_tensor(d[:], d[:], s[:], ALU.min)
                m = pool.tile([P, 1], F32, tag="fps_m")
                nc.vector.tensor_reduce(m[:], d[:], AX.X, ALU.max)
                mt = pool.tile([P, 32], F32, tag="fps_mt")
                nc.vector.transpose(mt[:], m[:, 0:1].broadcast_to((P, 32)))
                M = pool.tile([P, 1], F32, tag="fps_M")
                nc.vector.tensor_reduce(M[:], mt[:], AX.X, ALU.max)
                zp = pool.tile([P, 3 * F], F32, tag="fps_zp")
                d3 = d[:].unsqueeze(2).broadcast_to((P, F, 3))
                nc.vector.scalar_tensor_tensor(
                    zp[:].rearrange("p (f k) -> p f k", k=3), d3, M[:, 0:1],
                    payload3[:].rearrange("p (f k) -> p f k", k=3),
                    ALU.is_equal, ALU.mult)
                nc.vector.tensor_reduce(
                    out3pad[:, 0:3],
                    zp[:].rearrange("p (f k) -> p k f", k=3), AX.X, ALU.add)
                tr1 = pool.tile([P, 32], F32, tag="fps_tr1")
                nc.vector.transpose(tr1[:], out3pad[:])
                R = pool.tile([P, 1], F32, tag="fps_R")
                nc.vector.tensor_reduce(R[:], tr1[:], AX.X, ALU.add)
                vals = pool.tile([P, 32], F32, tag="fps_vals")
                nc.vector.transpose(vals[:], R[:, 0:1].broadcast_to((P, 32)))
                nc.vector.tensor_copy(wbuf[:, t:t + 1], vals[:, 0:1])

            for b in range(B):
                nc.sync.dma_start(fi_out.ap()[b:b + 1, :],
                                  wbuf[32 * b:32 * b + 1, :])

    _fix_sync_waits(nc)
    return _make_runner(nc)


_fps_runners = {}


def _fps_device(xy):
    """xy [4, N, 2] float32 -> fi [4, N//4] int64 via the bass kernel."""
    N = xy.shape[1]
    if N not in _fps_runners:
        _fps_runners[N] = _build_fps_runner(N)
    res = _fps_runners[N]({"xy": np.ascontiguousarray(xy, np.float32)})
    return res["fi"].astype(np.int64)




# ---------------------------------------------------------------------------
# KNN bass kernel: top-16 by score s = 2 q.c - |c|^2 via PE matmul + DVE
# max8/max_index/match_replace. Queries padded to multiples of 128.
# ---------------------------------------------------------------------------
U16 = mybir.dt.uint16


def _build_knn_runner(nq, nc_pts):
    """nq queries (mult of 128), nc_pts candidates (mult of 512)."""
    P = 128
    ncb = bass.Bass("TRN2", target_bir_lowering=False, debug=False,
                    num_devices=1)
    qT = ncb.dram_tensor("qT", [B, 3, nq], F32, kind="ExternalInput")
    cT = ncb.dram_tensor("cT", [B, 3, nc_pts], F32, kind="ExternalInput")
    idx_out = ncb.dram_tensor("idx", [B, nq, 16], U16, kind="ExternalOutput")

    nchunk = nc_pts // 512
    with tile.TileContext(ncb) as tc:
        with tc.tile_pool(name="p", bufs=3) as pool, \
             tc.tile_pool(name="cpool", bufs=1) as cpool, \
             tc.tile_pool(name="ps", bufs=4, space="PSUM") as psp:
            for b in range(B):
                ctile = cpool.tile([3, nc_pts], F32, tag="ctile")
                ncb.sync.dma_start(ctile[:], cT.ap()[b])
                for q0 in range(0, nq, P):
                    qtile = pool.tile([3, P], F32, tag="qtile")
                    ncb.sync.dma_start(qtile[:], qT.ap()[b][:, q0:q0 + P])
                    sc = pool.tile([P, nc_pts], F32, tag="sc")
                    for ch in range(nchunk):
                        ps = psp.tile([P, 512], F32, tag="ps")
                        ncb.tensor.matmul(ps[:], qtile[:],
                                          ctile[:, 512 * ch:512 * (ch + 1)],
                                          start=True, stop=True)
                        ncb.scalar.copy(sc[:, 512 * ch:512 * (ch + 1)], ps[:])
                    v8a = pool.tile([P, 8], F32, tag="v8a")
                    i8 = pool.tile([P, 16], U16, tag="i8")
                    ncb.vector.max(v8a[:], sc[:])
                    ncb.vector.max_index(i8[:, 0:8], v8a[:], sc[:])
                    ncb.vector.match_replace(sc[:], v8a[:], sc[:], -1e30)
                    v8b = pool.tile([P, 8], F32, tag="v8b")
                    ncb.vector.max(v8b[:], sc[:])
                    ncb.vector.max_index(i8[:, 8:16], v8b[:], sc[:])
                    ncb.sync.dma_start(idx_out.ap()[b][q0:q0 + P, :], i8[:])
    _fix_sync_waits(ncb)
    return _make_runner(ncb)


_knn_runners = {}


def _knn_device(q_xy, c_xy):
    """q_xy [B, Nq, 2], c_xy [B, Nc, 2] -> idx [B, Nq, 16] int64."""
    nq_real = q_xy.shape[1]
    nq = ((nq_real + 127) // 128) * 128
    nc_real = c_xy.shape[1]
    nc_pad = ((nc_real + 511) // 512) * 512
    key = (nq, nc_pad)
    if key not in _knn_runners:
        _knn_runners[key] = _build_knn_runner(nq, nc_pad)
    qT = np.zeros((B, 3, nq), np.float32)
    qT[:, 0, :nq_real] = 2.0 * q_xy[..., 0]
    qT[:, 1, :nq_real] = 2.0 * q_xy[..., 1]
    qT[:, 2, :nq_real] = 1.0
    cT = np.full((B, 3, nc_pad), 0.0, np.float32)
    cT[:, 0, :nc_real] = c_xy[..., 0]
    cT[:, 1, :nc_real] = c_xy[..., 1]
    # score for padded candidates must be very negative: with cx=cy=0 the
    # third row gives s = 1*val: set val=-1e30 for pad columns
    nrm = (c_xy[..., 0] * c_xy[..., 0] + c_xy[..., 1] * c_xy[..., 1]).astype(np.float32)
    cT[:, 2, :nc_real] = -nrm
    cT[:, 2, nc_real:] = -1e30
    res = _knn_runners[key]({"qT": qT, "cT": cT})
    idx = res["idx"][:, :nq_real, :].astype(np.int64)
    return idx


# ---------------------------------------------------------------------------
# dense remainder on the neuron cores via XLA
# ---------------------------------------------------------------------------
def _linear(p, x):
    return x @ p["W"] + p["b"]


def _gather(points, idx):
    b = idx.shape[0]
    flat = idx.reshape(b, -1)
    out = jnp.take_along_axis(points, flat[..., None], axis=1)
    return out.reshape(*idx.shape, points.shape[-1])


def _pre_stage(params, x):
    return _linear(params["fc1b"], jax.nn.relu(_linear(params["fc1a"], x)))


def _tf_stage(p, xy, feats, idx):
    knn_xy = _gather(xy, idx)
    x = _linear(p["fc1"], feats)
    q = x @ p["wq"]
    kk = _gather(x @ p["wk"], idx)
    v = _gather(x @ p["wv"], idx)
    pos = _linear(p["delta2"], jax.nn.relu(
        _linear(p["delta1"], xy[:, :, None] - knn_xy)))
    attn = _linear(p["gamma2"], jax.nn.relu(
        _linear(p["gamma1"], q[:, :, None] - kk + pos)))
    d = attn.shape[-1]
    attn = jax.nn.softmax(attn / jnp.sqrt(jnp.asarray(d, attn.dtype)),
                          axis=-2)
    res = jnp.einsum('bmnf,bmnf->bmf', attn, v + pos)
    return _linear(p["fc2"], res) + feats


def _td_stage(p, xy, points, new_xy, idx):
    gxn = _gather(xy, idx) - new_xy[:, :, None]
    feats = jnp.concatenate([gxn, _gather(points, idx)], -1)
    for c in p["convs"]:
        h = feats @ c["W"] + c["b"]
        mu = jnp.mean(h, axis=(0, 1, 2))
        var = jnp.var(h, axis=(0, 1, 2))
        h = (h - mu) * lax.rsqrt(var + 1e-5) * c["gamma"] + c["beta"]
        feats = jax.nn.relu(h)
    return jnp.max(feats, axis=2)


_jit_pre = jax.jit(_pre_stage)
_jit_tf = jax.jit(_tf_stage)
_jit_td = jax.jit(_td_stage)


# ---------------------------------------------------------------------------
# entry point
# ---------------------------------------------------------------------------
def kernel(x, params):
    x = np.asarray(x, np.float32)
    params = jax.tree.map(lambda a: np.asarray(a, np.float32), params)
    xy = x[..., :2]

    h = _jit_pre(params, jnp.asarray(x))
    idx1 = _knn_device(xy, xy)
    pts = _jit_tf(params["t1"], jnp.asarray(xy), h, jnp.asarray(idx1))
    outs = [jnp.asarray(xy), pts]
    for i, bp in enumerate(params["blocks"]):
        fi = _fps_device(xy)
        new_xy = np.take_along_axis(xy, fi[..., None], axis=1)
        idx_td = _knn_device(new_xy, xy)
        pts = _jit_td(bp["td"], jnp.asarray(xy), pts, jnp.asarray(new_xy),
                      jnp.asarray(idx_td))
        xy = new_xy
        idx_tf = _knn_device(xy, xy)
        pts = _jit_tf(bp["tf"], jnp.asarray(xy), pts, jnp.asarray(idx_tf))
        outs += [jnp.asarray(xy), pts]
    out = (pts, *outs)
    return tuple(np.asarray(o, np.float32) for o in out)
